# revision 13
# baseline (speedup 1.0000x reference)
"""Trainium2 Bass kernel for nn_AdaptiveCentralLayer (v8).

Input: kernel (128, 8, 256, 256) f32. Per (b, c) slice: compute center of
mass, then circularly roll the 256x256 slice so the center of mass lands
at the center (torch.roll semantics, per-slice data-dependent shifts).

Distribution: pure data parallel, batch dim sharded across 8 NeuronCores
(16 batches per core = 128 slices per core).

Per-core dataflow (v8 -- measured-rate-balanced engine assignment):
  1. One batched DMA load per G-slice group (row-interleaved: partition p
     holds image rows 2p, 2p+1; compact tile, 2 KB chunks, Act-issued).
  2. DVE pass 1 (per slice, per row-parity): fused tensor_scalar
     f32->bf16 cast copy with f32 accum -> row sums rs[p, g].
     (Act activation copies measured ~3x slower per element; PE
     data-as-weights LDWEIGHTS measured ~300 ns/chunk -- both rejected.)
  3. DVE pass 2 (per slice): fused tensor_tensor_reduce x-dot with a
     [P, 2W] column-weight constant -> per-partition x-moments xd[p].
  4. PE: two tiny single-row matmuls per group contract [rs0|rs1|xd]
     with ones / (2p-127.5) -> S, S1, X, Yt rows in one PSUM bank.
  5. Scalar stage batched over SUPER=4 groups (32 slices): Act copies the
     PSUM moment rows into one SBUF strip per super-group; DVE computes
     shifts and packs (row_off, col_off) int32 pairs; ~26 small-vector
     ops per 32 slices instead of per 8.
  6. Output: ONE DMA per slice (slices alternate SP/Act HWDGE queues;
     each engine batch-loads its 4 offset pairs per group in a single
     multi-value register load) writes the bf16 tile into a 512x512
     padded per-slice DRAM region at dynamic (row, col) offset -- the
     DMA performs BOTH circular rolls; nothing wraps. True out[r, w] =
     OR of the 4 region quadrant aliases (regions start zeroed, exactly
     one alias written); host folds with a uint16 OR, then upcasts.

Output precision: bf16 (L2 rel err 1.5e-3, gate 2e-2). Moments are f32:
min slice rounding margin is 7e-5; bf16-derived moments flip shifts,
f32-derived ones never do.

HBM traffic per core: 33.5 MB f32 read + 16.8 MB bf16 write = 50.3 MB
-> 140.5 us roofline at 358 GB/s per-core HBM.
"""
import numpy as np

import concourse.bass as bass
import concourse.bacc as bacc
import concourse.mybir as mybir
from concourse.tile import TileContext
from concourse.bass_utils import run_bass_kernel_spmd

B, C, H, W = 128, 8, 256, 256
NCORES = 8
BPC = B // NCORES            # batches per core
NS = BPC * C                 # slices per core
ROWS = NS * H                # true output rows per core
RREG = 512                   # padded rows per slice in the device region
W2 = 512                     # padded cols per slice in the device region
G = 8                        # slices per group
SUPER = 4                    # groups per scalar-stage batch
P = 128
F32 = mybir.dt.float32
BF16 = mybir.dt.bfloat16
I32 = mybir.dt.int32

SP = mybir.EngineType.SP
ACT = mybir.EngineType.Activation


def _build(ns=NS, repeat=1, dbufs=6, cbufs=28, g=G, sup=2):
    nc = bacc.Bacc("TRN2", target_bir_lowering=False, debug=False,
                   num_devices=NCORES)
    x = nc.dram_tensor("x", [ns * H, W], F32, kind="ExternalInput")
    out = nc.dram_tensor("out", [ns * RREG, W2], BF16, kind="ExternalOutput")
    cw = nc.dram_tensor("cw", [P, 2], F32, kind="ExternalInput")
    xct = nc.dram_tensor("xct", [P, 2 * W], F32, kind="ExternalInput")

    # row-interleaved: image row r = 2p + gg  (p = partition, gg = row LSB)
    x4 = x.rearrange("(s p gg) w -> s p gg w", p=P, gg=2)
    ov = out.rearrange("(s r) w -> s r w", r=RREG)
    ngrp = ns // g
    nsup = ngrp // sup

    with TileContext(nc) as tc:
        with (
            tc.tile_pool(name="consts", bufs=1) as kpool,
            tc.tile_pool(name="dpool", bufs=dbufs) as dpool,
            tc.tile_pool(name="cpool", bufs=cbufs) as cpool,
            tc.tile_pool(name="xsp", bufs=3) as xspool,
            tc.tile_pool(name="mpool", bufs=dbufs + 2) as mpool,
            tc.tile_pool(name="msup", bufs=2) as mspool,
            tc.tile_pool(name="spool", bufs=2) as spool,
            tc.tile_pool(name="psm", bufs=3, space="PSUM") as psmp,
        ):
            cw_t = kpool.tile([P, 2], F32)
            nc.sync.dma_start(out=cw_t[:], in_=cw[:])
            xct_t = kpool.tile([P, 2 * W], F32)
            nc.sync.dma_start(out=xct_t[:], in_=xct[:])

            def emit_group_compute(grp, msb, kk):
                """Load + per-slice DVE passes + moment matmuls + PSUM->SBUF
                copy into column block kk of the super-group strip msb.
                Returns the cb tiles for the later writeout."""
                Sb = grp * g
                d4 = dpool.tile([P, g, 2, W], F32, tag="d4")
                nc.scalar.dma_start(
                    out=d4[:],
                    in_=x4[Sb:Sb + g].transpose([1, 0, 2, 3]))

                m4 = mpool.tile([P, g, 3], F32, tag="m4")
                cb_tiles = []
                for s in range(g):
                    cb = cpool.tile([P, 2, W], BF16, tag="cb")
                    # fused bf16 cast + f32 row-sum accum (one per parity)
                    for gg in range(2):
                        nc.vector.tensor_scalar(
                            out=cb[:, gg, :], in0=d4[:, s, gg, :],
                            scalar1=1.0, scalar2=0.0,
                            op0=mybir.AluOpType.mult,
                            op1=mybir.AluOpType.add,
                            accum_out=m4[:, s, gg:gg + 1])
                    # fused x-dot over both halves (weights tiled 2x);
                    # scalar_tensor_tensor: out=(d*1.0)*xw, accum=sum(out)
                    # (tensor_tensor_reduce crashes the exec unit at
                    # runtime on this stack -- probed 2026-08-09)
                    xscr = xspool.tile([P, 2 * W], BF16, tag="xscr")
                    nc.vector.scalar_tensor_tensor(
                        out=xscr[:],
                        in0=d4[:, s].rearrange("p gg w -> p (gg w)"),
                        scalar=1.0, in1=xct_t[:],
                        op0=mybir.AluOpType.mult,
                        op1=mybir.AluOpType.mult,
                        accum_out=m4[:, s, 2:3])
                    cb_tiles.append(cb)

                # two tiny single-row matmuls -> one PSUM bank:
                # row A = ones^T [rs0|rs1|xd], row B = (2p-127.5)^T [...]
                m4f = m4[:].rearrange("p s q -> p (s q)")
                nq = 3 * g
                psM = psmp.tile([1, 2 * nq], F32, space="PSUM", tag="psM")
                nc.tensor.matmul(out=psM[0:1, 0:nq], lhsT=cw_t[:, 0:1],
                                 rhs=m4f, start=True, stop=True)
                nc.tensor.matmul(out=psM[0:1, nq:2 * nq], lhsT=cw_t[:, 1:2],
                                 rhs=m4f, start=True, stop=True)
                # stage the moment strip through SBUF (DVE may read at most
                # one PSUM operand; batched stage wants SBUF anyway)
                nc.scalar.copy(out=msb[0:1, kk * 2 * nq:(kk + 1) * 2 * nq],
                               in_=psM[:])
                return cb_tiles

            def emit_super(sg):
                nq = 3 * g
                msb = mspool.tile([1, sup * 2 * nq], F32, tag="msb")
                cbs = []
                for kk in range(sup):
                    cbs.extend(emit_group_compute(sg * sup + kk, msb, kk))

                # ---- batched scalar stage over sup*g slices ----
                # per group block: [S0 S1 X] * g  then  [Y0 Y1 junk] * g
                vA = msb[0:1, :].rearrange("o (k h s q) -> o k h s q",
                                           k=sup, h=2, q=3)
                vS = vA[:, :, 0]          # o k s q : ones-contracted
                vY = vA[:, :, 1]          # o k s q : w1-contracted

                def tt(t0, t1, tagp, op=mybir.AluOpType.add):
                    o = spool.tile([1, sup * g], F32, tag=tagp)
                    ov_ = o[0:1, :].rearrange("o (k s) -> o k s", k=sup)
                    nc.vector.tensor_tensor(out=ov_, in0=t0, in1=t1, op=op)
                    return o

                srow = tt(vS[:, :, :, 0], vS[:, :, :, 1], "srow")
                syp0 = tt(vY[:, :, :, 0], vY[:, :, :, 1], "syp0")
                srv = srow[0:1, :].rearrange("o (k s) -> o k s", k=sup)
                sypv = syp0[0:1, :].rearrange("o (k s) -> o k s", k=sup)
                # Y = Y0 + Y1 + S1   (row r = 2p+gg: (r-127.5) = w1 + gg)
                syp = spool.tile([1, sup * g], F32, tag="syp")
                nc.vector.tensor_tensor(
                    out=syp[0:1, :].rearrange("o (k s) -> o k s", k=sup),
                    in0=sypv, in1=vS[:, :, :, 1], op=mybir.AluOpType.add)
                sxp = spool.tile([1, sup * g], F32, tag="sxp")
                nc.vector.tensor_copy(
                    out=sxp[0:1, :].rearrange("o (k s) -> o k s", k=sup),
                    in_=vS[:, :, :, 2])

                rS = spool.tile([1, sup * g], F32, tag="rS")
                nc.vector.reciprocal(out=rS[:], in_=srow[:])

                # shift = floor(1 - mom/S) = round(0.5 - mom/S); floor from
                # int-cast + fix, correct under either cast rounding mode.
                def floor_shift(mom, tagp):
                    a = spool.tile([1, sup * g], F32, tag=tagp + "a")
                    nc.vector.tensor_tensor(out=a[:], in0=mom, in1=rS[:],
                                            op=mybir.AluOpType.mult)
                    nc.vector.tensor_scalar(out=a[:], in0=a[:],
                                            scalar1=-1.0, scalar2=1.0,
                                            op0=mybir.AluOpType.mult,
                                            op1=mybir.AluOpType.add)
                    fi = spool.tile([1, sup * g], I32, tag=tagp + "i")
                    nc.vector.tensor_copy(out=fi[:], in_=a[:])
                    fb = spool.tile([1, sup * g], F32, tag=tagp + "b")
                    nc.vector.tensor_copy(out=fb[:], in_=fi[:])
                    gt = spool.tile([1, sup * g], F32, tag=tagp + "g")
                    nc.vector.tensor_tensor(out=gt[:], in0=fb[:], in1=a[:],
                                            op=mybir.AluOpType.is_gt)
                    sf = spool.tile([1, sup * g], F32, tag=tagp + "s")
                    nc.vector.tensor_tensor(out=sf[:], in0=fb[:], in1=gt[:],
                                            op=mybir.AluOpType.subtract)
                    return sf

                syf = floor_shift(syp[:], "fy")
                sxf = floor_shift(sxp[:], "fx")

                # pack (row_off, col_off) = (sy & 255, sx & 255) pairs
                wox = spool.tile([1, 2 * sup * g], I32, tag="wox")
                for sf, off in ((syf, 0), (sxf, 1)):
                    dst = wox[0:1, :].rearrange("o (s t) -> o s t", t=2)
                    dst = dst[:, :, off]
                    nc.vector.tensor_copy(out=dst, in_=sf[:])
                    nc.vector.tensor_scalar(out=dst, in0=dst,
                                            scalar1=256, scalar2=None,
                                            op0=mybir.AluOpType.add)
                    nc.vector.tensor_scalar(out=dst, in0=dst,
                                            scalar1=255, scalar2=None,
                                            op0=mybir.AluOpType.bitwise_and)

                # ---- writeout: slices alternate SP/Act; each engine
                # multi-loads its 4 offset pairs per group in one go ----
                woxv = wox[0:1, :].rearrange("o (s t) -> o s t", t=2)
                hg = g // 2
                for kk in range(sup):
                    regs = {}
                    for half, engs_ in enumerate(([SP], [ACT])):
                        lo = kk * g + half * hg
                        sel = woxv[:, lo:lo + hg, :]
                        _, vals = nc.values_load_multi_w_load_instructions(
                            sel, engines=engs_,
                            min_val=0, max_val=255,
                            skip_runtime_bounds_check=True)
                        regs[half] = vals
                    for s in range(g):
                        S = (sg * sup + kk) * g + s
                        half, idx = (0, s) if s < hg else (1, s - hg)
                        eng = nc.sync if half == 0 else nc.scalar
                        w0 = regs[half][2 * idx]
                        ox = regs[half][2 * idx + 1]
                        ap = ov[S][bass.ds(w0, 2 * P), bass.ds(ox, W)]
                        ap = ap.rearrange("(p gg) w -> p gg w", gg=2, p=P)
                        eng.dma_start(out=ap, in_=cbs[kk * g + s][:])

            def emit_body():
                for sg in range(nsup):
                    emit_super(sg)

            if repeat == 1:
                emit_body()
            else:
                with tc.For_i(0, repeat, 1):
                    emit_body()

    nc.compile()
    return nc


def _consts():
    p = np.arange(P, dtype=np.float32)
    cw = np.stack([np.ones(P, np.float32),
                   (2.0 * p - 127.5).astype(np.float32)], axis=1)
    xw = (np.arange(W) - 127.5).astype(np.float32)
    xct = np.broadcast_to(np.tile(xw, 2), (P, 2 * W)).copy()
    return {"cw": cw, "xct": xct}


def fold_out(raw, ns=NS):
    """raw: [ns*RREG, W2] bf16 padded regions -> [ns*H, W] f32 true rows.

    Region cell (r, c) holds true cell (r mod 256, c mod 256); exactly one
    of the 4 aliases is written per true cell and the region starts
    zeroed, so a uint16 bitwise-or folds both wrap spills exactly."""
    r = raw.view(np.uint16).reshape(ns, 2, H, 2, W)
    merged = (r[:, 0, :, 0, :] | r[:, 0, :, 1, :]
              | r[:, 1, :, 0, :] | r[:, 1, :, 1, :])
    return merged.view(raw.dtype).astype(np.float32).reshape(ns * H, W)


_NC_CACHE = {}


def _get_nc():
    if "nc" not in _NC_CACHE:
        _NC_CACHE["nc"] = _build()
    return _NC_CACHE["nc"]


def _in_maps(k):
    consts = _consts()
    in_maps = []
    for c in range(NCORES):
        shard = k[c * BPC:(c + 1) * BPC].reshape(ROWS, W)
        m = {"x": shard}
        m.update(consts)
        in_maps.append(m)
    return in_maps


def kernel(**inputs):
    k = np.ascontiguousarray(np.asarray(inputs["kernel"], dtype=np.float32))
    assert k.shape == (B, C, H, W)
    nc = _get_nc()
    in_maps = _in_maps(k)
    res = run_bass_kernel_spmd(nc, in_maps, core_ids=list(range(NCORES)))
    outs = [fold_out(res.results[i]["out"]).reshape(BPC, C, H, W)
            for i in range(NCORES)]
    full = np.concatenate(outs, axis=0)
    return full
